# revision 9
# baseline (speedup 1.0000x reference)
"""AdditiveAttention TRN2 kernel (8-core SPMD, data-parallel with load balancing).

out[b,q,:] = softmax_k(mask(Wv . tanh(qf[b,q,:] + kf[b,k,:]))) @ values[b]

Strategy:
  - Each batch b only needs its first KP_b = ceil(vl_b/128)*128 keys (the rest
    are masked to -1e6 -> exp == 0 exactly in fp32, matching the reference).
  - Work unit = (batch, 64 q-rows). 32 half-batches, weighted by KP_b, are
    sorted and dealt to 8 cores x 4 slots; slot s has one compile-time key
    count K_s = max KP in that slot (shapes identical across cores -> SPMD).
  - Features tanh(qf+kf) are computed in [H=128 part, K free] layout: one ACT
    instruction per q-row (bias = qf column, add fused into the activation).
  - Scores land directly as [rows, K] in PSUM via one-hot-Wv sliding-window
    matmuls (lhsT = window of a [H, 2R] tile holding Wv at column R-1).
  - Softmax: DVE mask-add + reduce_max(negate) + ACT exp(bias=-max,
    accum_out=Z), then PE-transpose of e and PE attn@V, scaled by 1/Z.
"""

from contextlib import ExitStack

import numpy as np

NCORES = 8
B, Q, K, QS, KS, H, DV = 16, 128, 1024, 512, 512, 128, 512
R = 64          # q-rows per segment
S = 4           # segments per core
KC = 128        # key-count granularity


def _plan(valid_lens):
    vl = np.asarray(valid_lens).astype(np.int64)
    kp = np.maximum(KC, ((vl + KC - 1) // KC) * KC)
    kp = np.minimum(kp, K)
    halves = [(int(b), r0) for b in range(B) for r0 in (0, R)]
    halves.sort(key=lambda h: -int(kp[h[0]]))
    ks = [int(kp[halves[NCORES * s][0]]) for s in range(S)]
    assign = [[halves[NCORES * s + c] for s in range(S)] for c in range(NCORES)]
    return ks, assign


def _build(ks):
    import concourse.bass as bass
    import concourse.bacc as bacc
    import concourse.mybir as mybir
    import concourse.tile as tile

    f32 = mybir.dt.float32
    bf16 = mybir.dt.bfloat16
    Tanh = mybir.ActivationFunctionType.Tanh
    Exp = mybir.ActivationFunctionType.Exp

    nc = bacc.Bacc()

    wqt_d = nc.dram_tensor("wqt", [QS, H], f32, kind="ExternalInput")
    wkt_d = nc.dram_tensor("wkt", [KS, H], f32, kind="ExternalInput")
    twv_d = nc.dram_tensor("twv", [H, 2 * R], bf16, kind="ExternalInput")
    iden_d = nc.dram_tensor("iden", [R, R], f32, kind="ExternalInput")
    qt_d, kt_d, v_d, m_d, o_d = [], [], [], [], []
    for s in range(S):
        qt_d.append(nc.dram_tensor(f"qt{s}", [QS, R], f32, kind="ExternalInput"))
        kt_d.append(nc.dram_tensor(f"kt{s}", [KS, ks[s]], f32, kind="ExternalInput"))
        v_d.append(nc.dram_tensor(f"v{s}", [ks[s], DV], f32, kind="ExternalInput"))
        m_d.append(nc.dram_tensor(f"m{s}", [ks[s]], f32, kind="ExternalInput"))
        o_d.append(nc.dram_tensor(f"out{s}", [R, DV], f32, kind="ExternalOutput"))

    with tile.TileContext(nc) as tc, ExitStack() as ctx:
        consts = ctx.enter_context(tc.tile_pool(name="consts", bufs=1))
        big = ctx.enter_context(tc.tile_pool(name="big", bufs=2))
        feats = ctx.enter_context(tc.tile_pool(name="feats", bufs=4))
        work = ctx.enter_context(tc.tile_pool(name="work", bufs=2))
        small = ctx.enter_context(tc.tile_pool(name="small", bufs=3))
        ps_proj = ctx.enter_context(
            tc.tile_pool(name="ps_proj", bufs=1, space="PSUM"))
        ps_sc = ctx.enter_context(tc.tile_pool(name="ps_sc", bufs=1, space="PSUM"))
        ps_sm = ctx.enter_context(tc.tile_pool(name="ps_sm", bufs=1, space="PSUM"))
        ps_et = ctx.enter_context(tc.tile_pool(name="ps_et", bufs=2, space="PSUM"))

        wqt_sb = consts.tile([128, 4, H], f32, tag="wqt")
        nc.sync.dma_start(out=wqt_sb, in_=wqt_d[:].rearrange("(c p) h -> p c h", p=128))
        wkt_sb = consts.tile([128, 4, H], f32, tag="wkt")
        nc.sync.dma_start(out=wkt_sb, in_=wkt_d[:].rearrange("(c p) h -> p c h", p=128))
        twv_sb = consts.tile([H, 2 * R], bf16, tag="twv")
        nc.sync.dma_start(out=twv_sb, in_=twv_d[:])
        iden_sb = consts.tile([R, R], f32, tag="iden")
        nc.sync.dma_start(out=iden_sb, in_=iden_d[:])
        zeros_sb = consts.tile([128, 512], bf16, tag="zeros")
        nc.vector.memset(zeros_sb, 0.0)

        def absorb(ps_slice):
            # Zero-writing matmul that opens a PSUM accumulation group: it
            # takes over the cross-engine WAR waits so the real matmuls in
            # the group stay under walrus's 2-sync-wait limit.
            m = ps_slice.partition_size()
            n = ps_slice.free_size()
            nc.tensor.matmul(ps_slice, zeros_sb[:, :m], zeros_sb[:, :n],
                             start=True, stop=False, skip_group_check=True)

        for s in range(S):
            kseg = ks[s]
            nkc = kseg // 128
            ng = (kseg + 511) // 512

            qt_sb = work.tile([128, 4, R], f32, tag="qt")
            nc.sync.dma_start(out=qt_sb, in_=qt_d[s][:].rearrange("(c p) r -> p c r", p=128))
            kt_sb = big.tile([128, 4, K], f32, tag="kt")
            nc.sync.dma_start(
                out=kt_sb[:, :, :kseg],
                in_=kt_d[s][:].rearrange("(c p) k -> p c k", p=128))
            v_sb = big.tile([128, K // 128, DV], f32, tag="v")
            nc.sync.dma_start(
                out=v_sb[:, :nkc, :],
                in_=v_d[s][:].rearrange("(kc p) d -> p kc d", p=128))
            mask_sb = work.tile([R, K], f32, tag="mask")
            m_ap = m_d[s][:]
            m_bcast = bass.AP(
                tensor=m_ap.tensor, offset=m_ap.offset,
                ap=[[0, R]] + [list(a) for a in m_ap.ap])
            nc.sync.dma_start(out=mask_sb[:, :kseg], in_=m_bcast)

            # projections: qfT [H, R], kfT [H, kseg]
            qf_ps = ps_proj.tile([128, R], f32, tag="qf")
            absorb(qf_ps)
            for c in range(4):
                nc.tensor.matmul(qf_ps, wqt_sb[:, c, :], qt_sb[:, c, :],
                                 start=False, stop=(c == 3),
                                 skip_group_check=True)
            qf_sb = small.tile([128, R], f32, tag="qf_sb")
            nc.vector.tensor_copy(qf_sb, qf_ps)

            kf_ps = ps_proj.tile([128, K], f32, tag="kf")
            for g in range(ng):
                lo, hi = g * 512, min((g + 1) * 512, kseg)
                absorb(kf_ps[:, lo:hi])
                for c in range(4):
                    nc.tensor.matmul(kf_ps[:, lo:hi], wkt_sb[:, c, :],
                                     kt_sb[:, c, lo:hi],
                                     start=False, stop=(c == 3),
                                     skip_group_check=True)
            kf_sb = work.tile([128, K], f32, tag="kf_sb")
            nc.vector.tensor_copy(kf_sb[:, :kseg], kf_ps[:, :kseg])

            # scores: [R, kseg] accumulated over 64 one-hot matmuls
            sc_ps = ps_sc.tile([R, K], f32, tag="sc")
            for g in range(ng):
                lo, hi = g * 512, min((g + 1) * 512, kseg)
                absorb(sc_ps[:, lo:hi])
            for r in range(R):
                ft = feats.tile([128, K], bf16, tag="ft")
                nc.scalar.activation(out=ft[:, :kseg], in_=kf_sb[:, :kseg],
                                     func=Tanh, bias=qf_sb[:, r:r + 1])
                for g in range(ng):
                    lo, hi = g * 512, min((g + 1) * 512, kseg)
                    nc.tensor.matmul(
                        sc_ps[:, lo:hi], twv_sb[:, R - 1 - r:2 * R - 1 - r],
                        ft[:, lo:hi], start=False, stop=(r == R - 1),
                        skip_group_check=True)

            # masked softmax over keys
            sc_sb = work.tile([R, K], f32, tag="sc_sb")
            nc.vector.tensor_tensor(out=sc_sb[:, :kseg], in0=sc_ps[:, :kseg],
                                    in1=mask_sb[:, :kseg],
                                    op=mybir.AluOpType.add)
            negmax = small.tile([R, 1], f32, tag="negmax")
            nc.vector.reduce_max(negmax, sc_sb[:, :kseg],
                                 axis=mybir.AxisListType.X, negate=True)
            e_sb = work.tile([R, K], f32, tag="e_sb")
            zsum = small.tile([R, 1], f32, tag="zsum")
            nc.scalar.activation(out=e_sb[:, :kseg], in_=sc_sb[:, :kseg],
                                 func=Exp, bias=negmax, accum_out=zsum)
            rinv = small.tile([R, 1], f32, tag="rinv")
            nc.vector.reciprocal(rinv, zsum)

            # attn @ V : transpose e in 128-col chunks, accumulate into [R, DV]
            av_ps = ps_sm.tile([R, DV], f32, tag="av")
            absorb(av_ps)
            for kc in range(nkc):
                et_ps = ps_et.tile([128, R], f32, tag="et")
                nc.tensor.transpose(et_ps, e_sb[:, kc * 128:(kc + 1) * 128],
                                    iden_sb)
                et_sb = small.tile([128, R], f32, tag="et_sb")
                nc.vector.tensor_copy(et_sb, et_ps)
                nc.tensor.matmul(av_ps, et_sb, v_sb[:, kc, :],
                                 start=False, stop=(kc == nkc - 1),
                                 skip_group_check=True)

            out_sb = small.tile([R, DV], f32, tag="out_sb")
            nc.vector.tensor_scalar_mul(out_sb, av_ps, rinv)
            nc.sync.dma_start(out=o_d[s][:], in_=out_sb)

    nc.compile()
    nc.finalize()
    return nc


def _in_maps(ks, assign, queries, keys, values, Wq, Wk, Wv, valid_lens):
    import ml_dtypes

    vl = np.asarray(valid_lens).astype(np.int64)
    twv = np.zeros((H, 2 * R), dtype=np.float32)
    twv[:, R - 1] = np.asarray(Wv, dtype=np.float32)[0]
    base = {
        "wqt": np.ascontiguousarray(np.asarray(Wq, np.float32).T),
        "wkt": np.ascontiguousarray(np.asarray(Wk, np.float32).T),
        "twv": twv.astype(ml_dtypes.bfloat16),
        "iden": np.eye(R, dtype=np.float32),
    }
    queries = np.asarray(queries, np.float32)
    keys = np.asarray(keys, np.float32)
    values = np.asarray(values, np.float32)
    maps = []
    for c in range(NCORES):
        m = dict(base)
        for s in range(S):
            b, r0 = assign[c][s]
            kseg = ks[s]
            m[f"qt{s}"] = np.ascontiguousarray(queries[b, r0:r0 + R, :].T)
            m[f"kt{s}"] = np.ascontiguousarray(keys[b, :kseg, :].T)
            m[f"v{s}"] = np.ascontiguousarray(values[b, :kseg, :])
            msk = np.zeros(kseg, dtype=np.float32)
            msk[vl[b]:] = -1e6
            m[f"m{s}"] = msk
        maps.append(m)
    return maps


_last_results = None


def kernel(queries, keys, values, Wq, Wk, Wv, valid_lens):
    global _last_results
    from concourse.bass_utils import run_bass_kernel_spmd

    ks, assign = _plan(valid_lens)
    nc = _build(ks)
    in_maps = _in_maps(ks, assign, queries, keys, values, Wq, Wk, Wv, valid_lens)
    _last_results = run_bass_kernel_spmd(nc, in_maps, list(range(NCORES)))
    res = _last_results.results
    out = np.zeros((B, Q, DV), dtype=np.float32)
    for c in range(NCORES):
        for s in range(S):
            b, r0 = assign[c][s]
            out[b, r0:r0 + R, :] = res[c][f"out{s}"]
    return out


# revision 15
# speedup vs baseline: 15.8909x; 15.8909x over previous
"""AdditiveAttention TRN2 kernel (8-core SPMD, data-parallel with load balancing).

out[b,q,:] = softmax_k(mask(Wv . tanh(qf[b,q,:] + kf[b,k,:]))) @ values[b]

Strategy:
  - Each batch b only needs its first KP_b = ceil(vl_b/64)*64 keys (the rest
    are masked to -1e6 -> exp == 0 exactly in fp32, matching the reference).
  - Work unit = (batch, 32 q-rows). 64 quarter-batches, weighted by KP_b, are
    sorted and dealt to 8 cores x 8 slots; slot s has one compile-time key
    count K_s = max KP in that slot (shapes identical across cores -> SPMD),
    so per-core work is balanced and the instruction stream is shared.
  - Projections (fp32r matmuls) for all segments run in a prologue.
  - Features tanh(qf+kf) in [H=128 part, K free] layout: per-row DVE
    tensor_scalar_add (per-partition scalar = qf column, 2x mode) builds
    G=8 rows of sums, then one giant ACT tanh (bf16 out) per chunk
    amortizes the ~185ns/instr ACT overhead. ACT is the bottleneck engine
    (~123us/core of pure tanh at 128 lanes x 1.2GHz).
  - Scores land directly as [rows, K] fp32 in PSUM via one-hot-Wv
    sliding-window bf16 matmuls (lhsT = window of a [H, 2R] tile holding Wv
    at column R-1); the mask is pre-added by a rank-1 (ones x mask) matmul
    that also opens the PSUM accumulation group.
  - Softmax: DVE reduce_max(negate) from PSUM + ACT exp(bias=-max,
    accum_out=Z); attn@V via PE-transpose + fp32r matmuls, scaled by 1/Z.
  - Softmax/AV of segment s-1 is emitted mid-way through segment s's row
    loop so the exp never stalls ACT's feature stream.
"""

from contextlib import ExitStack

import numpy as np

NCORES = 8
B, Q, K, QS, KS, H, DV = 16, 128, 1024, 512, 512, 128, 512
R = 32          # q-rows per segment
S = 8           # segments per core
KC = 64         # key-count granularity


def _plan(valid_lens):
    vl = np.asarray(valid_lens).astype(np.int64)
    kp = np.maximum(KC, ((vl + KC - 1) // KC) * KC)
    kp = np.minimum(kp, K)
    halves = [(int(b), r0) for b in range(B) for r0 in range(0, Q, R)]
    halves.sort(key=lambda h: -int(kp[h[0]]))
    ks = [int(kp[halves[NCORES * s][0]]) for s in range(S)]
    assign = [[halves[NCORES * s + c] for s in range(S)] for c in range(NCORES)]
    # run the smallest slot first: the kernel's startup stall is the first
    # segment's full DMA->proj->copy->adds chain, which scales with K_s.
    perm = [S - 1, S - 2] + list(range(S - 2))
    ks = [ks[p] for p in perm]
    assign = [[a[p] for p in perm] for a in assign]
    return ks, assign


def _build(ks, niter=1):
    import concourse.bass as bass
    import concourse.bacc as bacc
    import concourse.mybir as mybir
    import concourse.tile as tile

    f32 = mybir.dt.float32
    f32r = mybir.dt.float32r
    bf16 = mybir.dt.bfloat16
    Tanh = mybir.ActivationFunctionType.Tanh
    Exp = mybir.ActivationFunctionType.Exp

    nc = bacc.Bacc()

    wqt_d = nc.dram_tensor("wqt", [QS, H], f32, kind="ExternalInput")
    wkt_d = nc.dram_tensor("wkt", [KS, H], f32r, kind="ExternalInput")
    twv_d = nc.dram_tensor("twv", [H, 2 * R], bf16, kind="ExternalInput")
    iden_d = nc.dram_tensor("iden", [R, R], f32, kind="ExternalInput")
    qt_d, kt_d, v_d, m_d, o_d = [], [], [], [], []
    for s in range(S):
        qt_d.append(nc.dram_tensor(f"qt{s}", [QS, R], f32, kind="ExternalInput"))
        kt_d.append(nc.dram_tensor(f"kt{s}", [KS, ks[s]], f32r, kind="ExternalInput"))
        v_d.append(nc.dram_tensor(f"v{s}", [ks[s], DV], f32r, kind="ExternalInput"))
        m_d.append(nc.dram_tensor(f"m{s}", [ks[s]], bf16, kind="ExternalInput"))
        o_d.append(nc.dram_tensor(f"out{s}", [R, DV], f32, kind="ExternalOutput"))

    with tile.TileContext(nc) as tc, ExitStack() as ctx:
        consts = ctx.enter_context(tc.tile_pool(name="consts", bufs=1))
        big = ctx.enter_context(tc.tile_pool(name="big", bufs=4))
        sums_p = ctx.enter_context(tc.tile_pool(name="sums", bufs=2))
        ftb_p = ctx.enter_context(tc.tile_pool(name="ftb", bufs=2))
        work = ctx.enter_context(tc.tile_pool(name="work", bufs=2))
        kfq = ctx.enter_context(tc.tile_pool(name="kfq", bufs=3))
        small = ctx.enter_context(tc.tile_pool(name="small", bufs=3))
        ps_proj = ctx.enter_context(
            tc.tile_pool(name="ps_proj", bufs=1, space="PSUM"))
        ps_sc = ctx.enter_context(tc.tile_pool(name="ps_sc", bufs=2, space="PSUM"))
        ps_sm = ctx.enter_context(tc.tile_pool(name="ps_sm", bufs=1, space="PSUM"))
        ps_et = ctx.enter_context(tc.tile_pool(name="ps_et", bufs=2, space="PSUM"))

        wqt_sb = consts.tile([128, 4, H], f32, tag="wqt")
        nc.sync.dma_start(out=wqt_sb, in_=wqt_d[:].rearrange("(c p) h -> p c h", p=128))
        wkt_sb = consts.tile([128, 4, H], f32r, tag="wkt")
        nc.sync.dma_start(out=wkt_sb, in_=wkt_d[:].rearrange("(c p) h -> p c h", p=128))
        twv_sb = consts.tile([H, 2 * R], bf16, tag="twv")
        nc.sync.dma_start(out=twv_sb, in_=twv_d[:])
        iden_sb = consts.tile([R, R], f32, tag="iden")
        nc.sync.dma_start(out=iden_sb, in_=iden_d[:])
        zeros_sb = consts.tile([128, 512], bf16, tag="zeros")
        nc.vector.memset(zeros_sb, 0.0)
        ones1 = consts.tile([1, R], bf16, tag="ones1")
        nc.vector.memset(ones1, 1.0)

        loop_cm = tc.For_i(0, niter, 1) if niter > 1 else None

        def absorb(ps_slice):
            # Zero-writing matmul that opens a PSUM accumulation group: it
            # takes over the cross-engine WAR waits so the real matmuls in
            # the group stay under walrus's 2-sync-wait limit.
            m = ps_slice.partition_size()
            n = ps_slice.free_size()
            nc.tensor.matmul(ps_slice, zeros_sb[:, :m], zeros_sb[:, :n],
                             start=True, stop=False, skip_group_check=True)

        if loop_cm is not None:
            loop_cm.__enter__()

        # ---- just-in-time prefetch: segment s+1's DMAs + projections are
        # emitted inside segment s's row loop, so the DMA stream and the DVE
        # copies spread across the whole kernel instead of serializing into
        # a prologue that starves ACT. kf is projected directly into the
        # same PSUM tile that later holds the scores (lifetimes disjoint),
        # which keeps the PSUM budget at 8 banks.
        kf_sbs, qf_sbs, seg_ps_t, seg_v = {}, {}, {}, {}

        kt_tiles = {}

        def prefetch_dma(s):
            kseg = ks[s]
            kt_sb = big.tile([128, 4, K], f32r, tag="bigbuf")
            for c in range(4):
                nc.sync.dma_start(
                    out=kt_sb[:, c, :kseg],
                    in_=kt_d[s][c * 128:(c + 1) * 128, :])
            qt_sb = work.tile([128, 4, R], f32, tag="qt")
            nc.sync.dma_start(out=qt_sb, in_=qt_d[s][:].rearrange("(c p) r -> p c r", p=128))
            kt_tiles[s] = (kt_sb, qt_sb)

        def prefetch_proj(s):
            kseg = ks[s]
            ng = (kseg + 511) // 512
            kt_sb, qt_sb = kt_tiles.pop(s)

            qf_ps = ps_proj.tile([128, R], f32, tag="qf")
            absorb(qf_ps)
            for c in range(4):
                nc.tensor.matmul(qf_ps, wqt_sb[:, c, :], qt_sb[:, c, :],
                                 start=False, stop=(c == 3),
                                 skip_group_check=True)
            qf_sb = kfq.tile([128, R], f32, tag="qf_sb")
            nc.vector.tensor_copy(qf_sb, qf_ps)
            qf_sbs[s] = qf_sb

            seg_ps = ps_sc.tile([128, K], f32, tag="kfsc")
            seg_ps_t[s] = seg_ps
            for g in range(ng):
                lo, hi = g * 512, min((g + 1) * 512, kseg)
                absorb(seg_ps[:, lo:hi])
                for c in range(4):
                    nc.tensor.matmul(seg_ps[:, lo:hi],
                                     wkt_sb[:, c, :],
                                     kt_sb[:, c, lo:hi],
                                     start=False, stop=(c == 3),
                                     skip_group_check=True)
            kf_sb = kfq.tile([128, K], bf16, tag="kf_sb")
            nc.vector.tensor_copy(kf_sb[:, :kseg], seg_ps[:, :kseg])
            kf_sbs[s] = kf_sb

        def prefetch(s):
            prefetch_dma(s)
            prefetch_proj(s)

        seg_state = {}

        def tail(s):
            kseg = ks[s]
            nkc = (kseg + 127) // 128
            nfull = kseg // 128
            wlast = kseg - nfull * 128
            sc_ps, v_sb = seg_state[s]
            negmax = small.tile([R, 1], f32, tag="negmax")
            nc.vector.reduce_max(negmax, sc_ps[:R, :kseg],
                                 axis=mybir.AxisListType.X, negate=True)
            e_sb = work.tile([R, K], f32, tag="e_sb")
            zsum = small.tile([R, 1], f32, tag="zsum")
            nc.scalar.activation(out=e_sb[:, :kseg], in_=sc_ps[:R, :kseg],
                                 func=Exp, bias=negmax, accum_out=zsum)
            rinv = small.tile([R, 1], f32, tag="rinv")
            nc.vector.reciprocal(rinv, zsum)
            av_ps = ps_sm.tile([R, DV], f32, tag="av")
            absorb(av_ps)
            for kc in range(nkc):
                w = 128 if kc < nfull else wlast
                et_ps = ps_et.tile([128, R], f32, tag="et")
                nc.tensor.transpose(et_ps[:w, :],
                                    e_sb[:, kc * 128:kc * 128 + w], iden_sb)
                et_sb = small.tile([128, R], f32r, tag="et_sb")
                nc.vector.tensor_copy(et_sb[:w, :], et_ps[:w, :])
                nc.tensor.matmul(av_ps, et_sb[:w, :], v_sb[:w, kc, :],
                                 start=False, stop=(kc == nkc - 1),
                                 skip_group_check=True)
            out_sb = small.tile([R, DV], f32, tag="out_sb")
            nc.vector.tensor_scalar_mul(out_sb, av_ps, rinv)
            nc.sync.dma_start(out=o_d[s][:], in_=out_sb)

        prefetch(0)
        prefetch(1)
        kf_sbs[1] = kf_sbs[1]  # segments 0,1 fully prefetched up front
        for s in range(S):
            kseg = ks[s]
            nfull = kseg // 128
            wlast = kseg - nfull * 128
            ng = (kseg + 511) // 512
            kf_sb, qf_sb = kf_sbs[s], qf_sbs[s]

            v_sb = big.tile([128, K // 128, DV], f32r, tag="bigbuf")
            if nfull:
                nc.sync.dma_start(
                    out=v_sb[:, :nfull, :],
                    in_=v_d[s][:nfull * 128, :].rearrange(
                        "(kc p) d -> p kc d", p=128))
            if wlast:
                nc.sync.dma_start(out=v_sb[:wlast, nfull, :],
                                  in_=v_d[s][nfull * 128:, :])
            mask_sb = small.tile([1, K], bf16, tag="mask1")
            nc.sync.dma_start(out=mask_sb[:1, :kseg], in_=m_d[s][:])

            sc_ps = seg_ps_t[s]
            seg_state[s] = (sc_ps, v_sb)
            for g in range(ng):
                lo, hi = g * 512, min((g + 1) * 512, kseg)
                nc.tensor.matmul(sc_ps[:R, lo:hi], ones1,
                                 mask_sb[:1, lo:hi], start=True, stop=False,
                                 skip_group_check=True)
            G = 8
            for r0g in range(0, R, G):
                if r0g == 8 and s > 0:
                    tail(s - 1)
                    if s + 1 < S and s + 1 not in kf_sbs:
                        prefetch_proj(s + 1)
                if r0g == 16 and s + 2 < S:
                    prefetch_dma(s + 2)
                sums = sums_p.tile([128, 8 * K], bf16, tag="sums")
                for rl in range(G):
                    nc.vector.tensor_scalar_add(
                        sums[:, rl * kseg:(rl + 1) * kseg], kf_sb[:, :kseg],
                        qf_sb[:, r0g + rl:r0g + rl + 1])
                ft = ftb_p.tile([128, 8 * K], bf16, tag="ftb")
                nc.scalar.activation(out=ft[:, :G * kseg],
                                     in_=sums[:, :G * kseg], func=Tanh)
                for rl in range(G):
                    r = r0g + rl
                    for g in range(ng):
                        lo, hi = g * 512, min((g + 1) * 512, kseg)
                        nc.tensor.matmul(
                            sc_ps[:R, lo:hi],
                            twv_sb[:, R - 1 - r:2 * R - 1 - r],
                            ft[:, rl * kseg + lo:rl * kseg + hi],
                            start=False, stop=(r == R - 1),
                            skip_group_check=True)

        tail(S - 1)
        if loop_cm is not None:
            loop_cm.__exit__(None, None, None)

    nc.compile()
    nc.finalize()
    return nc


def _in_maps(ks, assign, queries, keys, values, Wq, Wk, Wv, valid_lens):
    import ml_dtypes

    vl = np.asarray(valid_lens).astype(np.int64)
    twv = np.zeros((H, 2 * R), dtype=np.float32)
    twv[:, R - 1] = np.asarray(Wv, dtype=np.float32)[0]
    base = {
        "wqt": np.ascontiguousarray(np.asarray(Wq, np.float32).T),
        "wkt": np.ascontiguousarray(np.asarray(Wk, np.float32).T),
        "twv": twv.astype(ml_dtypes.bfloat16),
        "iden": np.eye(R, dtype=np.float32),
    }
    queries = np.asarray(queries, np.float32)
    keys = np.asarray(keys, np.float32)
    values = np.asarray(values, np.float32)
    maps = []
    for c in range(NCORES):
        m = dict(base)
        for s in range(S):
            b, r0 = assign[c][s]
            kseg = ks[s]
            m[f"qt{s}"] = np.ascontiguousarray(queries[b, r0:r0 + R, :].T)
            m[f"kt{s}"] = np.ascontiguousarray(keys[b, :kseg, :].T)
            m[f"v{s}"] = np.ascontiguousarray(values[b, :kseg, :])
            msk = np.zeros(kseg, dtype=np.float32)
            msk[vl[b]:] = -1e6
            m[f"m{s}"] = msk.astype(ml_dtypes.bfloat16)
        maps.append(m)
    return maps


_last_results = None


def _devices_available():
    try:
        import jax
        devs = jax.devices()
        return len(devs) >= NCORES and devs[0].platform != "cpu"
    except Exception:
        return False


def _kernel_subprocess(queries, keys, values, Wq, Wk, Wv, valid_lens):
    # The calling process pinned jax to cpu (common when the reference runs
    # in-process); re-run in a child with the accelerator platform visible.
    import os
    import subprocess
    import sys
    import tempfile

    with tempfile.TemporaryDirectory() as td:
        inp, outp = os.path.join(td, "in.npz"), os.path.join(td, "out.npz")
        np.savez(inp, queries=queries, keys=keys, values=values,
                 Wq=Wq, Wk=Wk, Wv=Wv, valid_lens=valid_lens)
        env = dict(os.environ)
        env.pop("JAX_PLATFORMS", None)
        env["_KERNEL_NO_SUBPROC"] = "1"
        code = (
            "import numpy as np, importlib.util, sys\n"
            f"spec = importlib.util.spec_from_file_location('kmod', {__file__!r})\n"
            "m = importlib.util.module_from_spec(spec); spec.loader.exec_module(m)\n"
            f"d = np.load({inp!r})\n"
            "out = m.kernel(**{k: d[k] for k in d.files})\n"
            f"np.savez({outp!r}, out=out)\n"
        )
        subprocess.run([sys.executable, "-c", code], env=env, check=True)
        return np.load(outp)["out"]


def kernel(queries, keys, values, Wq, Wk, Wv, valid_lens):
    global _last_results
    import os
    # NTFF trace hooks don't exist on this axon client; a stray BASS_TRACE=1
    # in the environment would crash run_bass_kernel_spmd otherwise.
    os.environ["BASS_NEVER_TRACE"] = "1"
    if not os.environ.get("_KERNEL_NO_SUBPROC") and not _devices_available():
        return _kernel_subprocess(queries, keys, values, Wq, Wk, Wv,
                                  valid_lens)
    from concourse.bass_utils import run_bass_kernel_spmd

    ks, assign = _plan(valid_lens)
    nc = _build(ks)
    in_maps = _in_maps(ks, assign, queries, keys, values, Wq, Wk, Wv, valid_lens)
    _last_results = run_bass_kernel_spmd(nc, in_maps, list(range(NCORES)))
    res = _last_results.results
    out = np.zeros((B, Q, DV), dtype=np.float32)
    for c in range(NCORES):
        for s in range(S):
            b, r0 = assign[c][s]
            out[b, r0:r0 + R, :] = res[c][f"out{s}"]
    return out

